# revision 6
# baseline (speedup 1.0000x reference)
"""CASSI shear kernel for Trainium2 (Bass/Tile), 8-core SPMD.

Computes, for full inputs x (1, 1024, 1024, 31) and ca (1, 1024, 1024, 1):
    y1[m, n, l] = x[m, n, l] * ca[m, n]
    out[m, j]   = sum_{n+l=j} y1[m, n, l]       (j in [0, 1054))
returning (1, 1024, 1054, 1) float32.

Sharding: rows m across 8 cores (128 rows/core = one full SBUF partition
block). Per core, free dim holds the (n, l) plane contiguously (n-major,
matching HBM layout so DMA loads are fully contiguous per partition).

Compute split per 128-column chunk:
  - broadcast multiply x *= ca via zero-stride AP on the vector engine
    (first DVE_COLS columns) and gpsimd (rest) running concurrently;
  - the 31-way shear accumulation is ONE vector-engine tensor_tensor add
    per chunk, using an overlapping output access pattern
    acc[n0 + l + n] += y1[l, n] (n innermost, so revisits of the same
    address are CHUNK-1 cycles apart -- far beyond the 8-stage DVE pipe).
"""

import sys

import numpy as np

if "/opt/trn_rl_repo" not in sys.path:
    sys.path.insert(0, "/opt/trn_rl_repo")

M, N, L = 1024, 1024, 31
ONC = N + L - 1  # 1054
NCORES = 8
R = M // NCORES  # 128 rows per core
CHUNK = 128
NCHUNKS = N // CHUNK
DVE_COLS = 44  # columns of each chunk multiplied on DVE; rest on gpsimd

_cached_nc = {}


def _build_nc(loop_iters=None):
    """Build the per-core Bass program. loop_iters wraps the body in an
    on-device For_i that repeats the full computation (for benchmarking);
    None means run the body once, straight-line."""
    key = loop_iters
    if key in _cached_nc:
        return _cached_nc[key]

    import concourse.bass as bass
    import concourse.mybir as mybir
    from concourse import bacc
    from concourse.tile import TileContext

    f32 = mybir.dt.float32
    nc = bacc.Bacc("TRN2")

    xin = nc.dram_tensor("x", (R, N * L), f32, kind="ExternalInput")
    cain = nc.dram_tensor("ca", (R, N), f32, kind="ExternalInput")
    outd = nc.dram_tensor("out", (R, ONC), f32, kind="ExternalOutput")

    with TileContext(nc) as tc:
        with (
            tc.tile_pool(name="xp", bufs=4) as xp,
            tc.tile_pool(name="cp", bufs=1) as cp,
            tc.tile_pool(name="accp", bufs=1) as accp,
        ):
            ca_t = cp.tile([R, N], f32)
            nc.sync.dma_start(out=ca_t[:], in_=cain[:])

            # The TensorTensor encoding only has room for ONE sync wait;
            # Bacc spills extras onto EventSemaphore nops, but structure
            # deps to mostly avoid needing that:
            #  - memset acc on Pool (its completion rides the same Pool sem
            #    the shear-add already waits on for the gpsimd mul);
            #  - "touch" ca on each compute engine up front so the ca-DMA
            #    wait is absorbed before the first chunk's muls.
            acc = accp.tile([R, ONC], f32)

            scr0 = cp.tile([R, 1], f32, tag="scr0")
            scr1 = cp.tile([R, 1], f32, tag="scr1")
            nc.gpsimd.tensor_copy(scr0[:], ca_t[:, 0:1])
            nc.vector.tensor_copy(scr1[:], ca_t[:, 0:1])

            def body():
                nc.gpsimd.memset(acc[:], 0.0)
                for i in range(NCHUNKS):
                    n0 = i * CHUNK
                    xt = xp.tile([R, CHUNK * L], f32, tag="xchunk")
                    nc.sync.dma_start(
                        out=xt[:], in_=xin[:, n0 * L : (n0 + CHUNK) * L]
                    )

                    # [R, CHUNK, L] view of the chunk (n-major, l contiguous)
                    x3 = xt[:].rearrange("p (n l) -> p n l", l=L)
                    cab = (
                        ca_t[:, n0 : n0 + CHUNK]
                        .unsqueeze(2)
                        .broadcast_to([R, CHUNK, L])
                    )

                    d = DVE_COLS
                    nc.vector.tensor_tensor(
                        x3[:, 0:d, :], x3[:, 0:d, :], cab[:, 0:d, :],
                        mybir.AluOpType.mult,
                    )
                    nc.gpsimd.tensor_tensor(
                        x3[:, d:, :], x3[:, d:, :], cab[:, d:, :],
                        mybir.AluOpType.mult,
                    )

                    # Shear accumulate: acc[:, n0 + l + n] += y1[:, l, n]
                    ylm = xt[:].rearrange("p (n l) -> p l n", l=L)
                    av = acc[:, n0 : n0 + CHUNK]
                    part = [int(av.ap[0][0]), int(av.ap[0][1])]
                    sh = bass.AP(av.tensor, av.offset, [part, [1, L], [1, CHUNK]])
                    nc.vector.tensor_tensor(sh, ylm, sh, mybir.AluOpType.add)

                nc.sync.dma_start(out=outd[:], in_=acc[:])

            if loop_iters is None:
                body()
            else:
                with tc.For_i(0, loop_iters, 1):
                    body()

    nc.finalize()
    _cached_nc[key] = nc
    return nc


def _run(x_slab, ca_slab, loop_iters=None, **run_kwargs):
    """x_slab (M, N*L) f32, ca_slab (M, N) f32 -> (M, ONC) f32."""
    from concourse.bass_utils import run_bass_kernel_spmd

    nc = _build_nc(loop_iters)
    in_maps = []
    for c in range(NCORES):
        in_maps.append(
            {
                "x": np.ascontiguousarray(x_slab[c * R : (c + 1) * R]),
                "ca": np.ascontiguousarray(ca_slab[c * R : (c + 1) * R]),
            }
        )
    res = run_bass_kernel_spmd(nc, in_maps, core_ids=list(range(NCORES)), **run_kwargs)
    out = np.concatenate(
        [np.asarray(res.results[c]["out"]) for c in range(NCORES)], axis=0
    )
    return out, res


def kernel(x, ca):
    x = np.ascontiguousarray(np.asarray(x, dtype=np.float32).reshape(M, N * L))
    ca = np.ascontiguousarray(np.asarray(ca, dtype=np.float32).reshape(M, N))
    out, _ = _run(x, ca)
    return out.reshape(1, M, ONC, 1)
